# revision 14
# baseline (speedup 1.0000x reference)
"""Trainium2 Bass kernel for multi-head attention (B=4, T=2048, C=1024, H=16).

Sharding: 8 cores = (batch b in 0..3) x (head-group g in 0..1, 8 heads each).
Per core: QKV projections for its 512 dims, attention for 8 heads, partial
output projection. Host sums the two per-batch partials and adds the biases
that fold out of the device computation:
  - bk drops entirely (softmax is invariant to per-query additive constants)
  - bv folds to host:   out += Wo @ bv   (softmax rows sum to 1)
  - bo added on host
  - bq is applied on-device in the Q-projection drain (pre-scaled by host)

Numerics/engine plan (all projections bf16):
  - scores S^T = K^T.T Q^T per head pair, row-packed on the PE (the two
    64-contraction matmuls land in row groups 0/64 and run concurrently)
  - exp alternates whole key-chunks between ACT (exact Exp) and DVE
    (Schraudolph bit-trick: x -> int16(A*x+B) reinterpreted as bf16), so
    each psc PSUM buffer is freed by a single instruction
  - V is ones-augmented (cols 0:64 ones), so PV rows 0:63 hold the softmax
    denominator aligned for the custom-op reciprocal; muls read the data
    rows 64:128 with a standard-op partition shift
  - normalization on DVE, PSUM->SBUF drains split ACT/DVE, out-projection
    per query block, host adds partials
"""
import numpy as np
import ml_dtypes

import concourse.bass as bass
import concourse.mybir as mybir
import concourse.tile as tile
from concourse import bacc

F32 = mybir.dt.float32
BF16 = mybir.dt.bfloat16
I16 = mybir.dt.int16
AF = mybir.ActivationFunctionType
ALU = mybir.AluOpType

B, T, C = 4, 2048, 1024
H, CH = 16, 64
G = 512            # dims per head-group (8 heads)
NCIN = 8           # 128-chunks of C
NCOUT = 4          # 128-chunks of G
NTB = 4            # 512-wide t blocks
NKC = 16           # 128-wide key chunks
NQB = 4            # 512-wide query blocks
SCALE = 1.0 / np.sqrt(CH)

# Schraudolph exp: exp(x) ~= bitcast_bf16(int16(A_EXP*x + B_EXP)).
# Calibrated on the real logit distribution (std ~0.46); rel-err std ~1.8%,
# mean offset cancels in the softmax normalization.
A_EXP = float(2.0 ** 7 / np.log(2.0))
B_EXP = 16248.0
DVE_KC = (1, 5, 10, 14)    # key chunks exp'd on DVE (approx)
POOL_KC = (7, 12)          # key chunks exp'd on Pool (approx)


def build_nc(debug=False):
    nc = bacc.Bacc()
    xq = nc.declare_dram_parameter("xq", [C, T], BF16, isOutput=False)
    xk = nc.declare_dram_parameter("xk", [C, T], BF16, isOutput=False)
    xv = nc.declare_dram_parameter("xv", [C, T], BF16, isOutput=False)
    wq = nc.declare_dram_parameter("wq", [C, G], BF16, isOutput=False)
    wk = nc.declare_dram_parameter("wk", [C, G], BF16, isOutput=False)
    wv = nc.declare_dram_parameter("wv", [C, G], BF16, isOutput=False)
    wo = nc.declare_dram_parameter("wo", [G, C], BF16, isOutput=False)
    bq = nc.declare_dram_parameter("bq", [128, NCOUT], F32, isOutput=False)
    out = nc.declare_dram_parameter("out", [T, C], F32, isOutput=True)

    xq_r = xq.rearrange("(c p) t -> p c t", p=128)
    xk_r = xk.rearrange("(c p) t -> p c t", p=128)
    xv_r = xv.rearrange("(c p) t -> p c t", p=128)
    wk_r = wk.rearrange("(c p) g -> p c g", p=128)
    wq_r = wq.rearrange("(c p) g -> p c g", p=128)
    wv_r = wv.rearrange("(c p) g -> p c g", p=128)

    with tile.TileContext(nc) as tc:
        with tc.tile_pool(name="persist", bufs=1) as persist:
            qt = [persist.tile([128, T], BF16, tag=f"qt{i}", name=f"qt{i}")
                  for i in range(NCOUT)]
            kt = [persist.tile([128, T], BF16, tag=f"kt{i}", name=f"kt{i}")
                  for i in range(NCOUT)]
            # V augmented: per-head columns 0:64 are ones -> PV rows 0:63
            # all hold the softmax denominator, aligned for the custom-op
            # reciprocal (which reads inputs at the OUT base partition).
            # Whole-tile memset (contiguous); V-proj drains fill cols 64:128.
            v_aug = persist.tile([128, NKC, 8, 128], BF16, tag="vaug")
            nc.gpsimd.memset(v_aug[:, :, :, :], 1.0)
            wo_sb = persist.tile([128, NCOUT, C], BF16, tag="wo")
            nc.scalar.dma_start(
                out=wo_sb, in_=wo.rearrange("(c p) g -> p c g", p=128))

            # ---------- scope 1: K and Q projections (bf16) ----------
            with tc.tile_pool(name="wkq", bufs=1) as wkq, \
                 tc.tile_pool(name="xsA", bufs=2) as xsA, \
                 tc.tile_pool(name="psA", bufs=4, space="PSUM") as psA:
                wk_sb = wkq.tile([128, NCIN, G], BF16, tag="wk")
                wq_sb = wkq.tile([128, NCIN, G], BF16, tag="wq")
                bq_sb = wkq.tile([128, NCOUT], F32, tag="bq")
                for ci in range(NCIN):
                    nc.scalar.dma_start(out=wk_sb[:, ci, :],
                                        in_=wk_r[:, ci, :])
                for tb in range(NTB):
                    xk_t = xsA.tile([128, NCIN, 512], BF16, tag="xstream")
                    for ci in range(NCIN):
                        nc.default_dma_engine.dma_start(
                            out=xk_t[:, ci, :],
                            in_=xk_r[:, ci, tb * 512:(tb + 1) * 512])
                    for co in range(NCOUT):
                        ps = psA.tile([128, 512], F32, tag="psA")
                        for ci in range(NCIN):
                            nc.tensor.matmul(
                                ps, wk_sb[:, ci, co * 128:(co + 1) * 128],
                                xk_t[:, ci, :],
                                start=(ci == 0), stop=(ci == NCIN - 1))
                        nc.vector.tensor_copy(
                            out=kt[co][:, tb * 512:(tb + 1) * 512], in_=ps)
                for ci in range(NCIN):
                    nc.scalar.dma_start(out=wq_sb[:, ci, :],
                                        in_=wq_r[:, ci, :])
                nc.scalar.dma_start(out=bq_sb, in_=bq[:, :])
                for tb in range(NTB):
                    xq_t = xsA.tile([128, NCIN, 512], BF16, tag="xstream")
                    for ci in range(NCIN):
                        nc.default_dma_engine.dma_start(
                            out=xq_t[:, ci, :],
                            in_=xq_r[:, ci, tb * 512:(tb + 1) * 512])
                    for co in range(NCOUT):
                        ps = psA.tile([128, 512], F32, tag="psA")
                        for ci in range(NCIN):
                            nc.tensor.matmul(
                                ps, wq_sb[:, ci, co * 128:(co + 1) * 128],
                                xq_t[:, ci, :],
                                start=(ci == 0), stop=(ci == NCIN - 1))
                        # wq/bq pre-scaled by 1/sqrt(dh) on the host
                        nc.scalar.activation(
                            qt[co][:, tb * 512:(tb + 1) * 512], ps,
                            AF.Identity, bias=bq_sb[:, co:co + 1])

            # ---------- scope 2: V projection overlapped with attention ----
            with tc.tile_pool(name="wv2", bufs=1) as wv2, \
                 tc.tile_pool(name="xsV", bufs=2) as xsV, \
                 tc.tile_pool(name="eb", bufs=6) as eb, \
                 tc.tile_pool(name="otp", bufs=2) as otp, \
                 tc.tile_pool(name="dv", bufs=2) as dv, \
                 tc.tile_pool(name="ojp", bufs=2) as ojp, \
                 tc.tile_pool(name="scp", bufs=2, space="PSUM") as scp, \
                 tc.tile_pool(name="pvp", bufs=2, space="PSUM") as pvp:
                wv_sb = wv2.tile([128, NCIN, G], BF16, tag="wv")
                for ci in range(NCIN):
                    nc.scalar.dma_start(out=wv_sb[:, ci, :],
                                        in_=wv_r[:, ci, :])

                # V projection, interleaved into the first attention
                # steps (one key-chunk ahead of its PV consumer)
                xv_tiles = {}

                def emit_vproj(tcix):
                    tb, sub = divmod(tcix, 4)
                    if sub == 0:
                        xv_t = xsV.tile([128, NCIN, 512], BF16,
                                        tag="xvstream", name="xv_t")
                        for ci in range(NCIN):
                            nc.default_dma_engine.dma_start(
                                out=xv_t[:, ci, :],
                                in_=xv_r[:, ci, tb * 512:(tb + 1) * 512])
                        xv_tiles[tb] = xv_t
                    xv_t = xv_tiles[tb]
                    ps2 = scp.tile([128, 2, 512], F32, tag="sc", name="psv")
                    ps = ps2[:, 0, :]
                    for ci in range(NCIN):
                        nc.tensor.matmul(
                            ps, xv_t[:, ci, sub * 128:(sub + 1) * 128],
                            wv_sb[:, ci, :],
                            start=(ci == 0), stop=(ci == NCIN - 1))
                    if tcix % 2 == 0:
                        nc.vector.tensor_copy(
                            out=v_aug[:, tcix, :, 64:128], in_=ps)
                    else:
                        nc.scalar.copy(
                            out=v_aug[:, tcix, :, 64:128], in_=ps)

                # attention, software-pipelined: scores/exp run LOOK steps
                # ahead of the PV accumulation over the flat (qb,p,kc) stream
                NSTEP = NQB * NCOUT * NKC
                LOOK = 2
                e_t, pv_t, ot_t = {}, {}, {}

                def sk(i):
                    return i >> 6, (i >> 4) & 3, i & 15

                def emit_scores_exp(i):
                    qb, p, kc = sk(i)
                    qsl = slice(qb * 512, (qb + 1) * 512)
                    ksl = slice(kc * 128, (kc + 1) * 128)
                    psc = scp.tile([128, 2, 512], F32, tag="sc", name="psc")
                    nc.tensor.matmul(psc[:, 0, :], kt[p][0:64, ksl],
                                     qt[p][0:64, qsl], start=True, stop=True)
                    nc.tensor.matmul(psc[:, 1, :], kt[p][64:128, ksl],
                                     qt[p][64:128, qsl], start=True, stop=True)
                    e = eb.tile([128, 2, 512], BF16, tag="e", name="e")
                    # whole-kc exp on a single engine so the psc buffer is
                    # freed by one instruction (decouples the ACT/DVE queues):
                    # odd kc -> DVE Schraudolph approx, else ACT exact
                    if kc % 2 == 1:
                        nc.vector.tensor_scalar(
                            e.bitcast(I16), psc, A_EXP, B_EXP,
                            ALU.mult, ALU.add)
                    else:
                        nc.scalar.activation(e, psc, AF.Exp)
                    e_t[i] = e

                def emit_pv(i):
                    qb, p, kc = sk(i)
                    if kc == 0:
                        pv_t[(qb, p)] = pvp.tile([128, 2, 512], F32,
                                                 tag="pv", name="pv")
                    pv = pv_t[(qb, p)]
                    e = e_t.pop(i)
                    nc.tensor.matmul(
                        pv[:, 0, :], v_aug[:, kc, 2 * p, :], e[:, 0, :],
                        start=(kc == 0), stop=(kc == NKC - 1))
                    nc.tensor.matmul(
                        pv[:, 1, :], v_aug[:, kc, 2 * p + 1, :], e[:, 1, :],
                        start=(kc == 0), stop=(kc == NKC - 1))

                def emit_normalize(qb, p):
                    if qb not in ot_t:
                        ot_t[qb] = otp.tile([128, NCOUT, 512], BF16,
                                            tag="ot", name="ot")
                    ot = ot_t[qb]
                    pv = pv_t.pop((qb, p))
                    # denominators sit on partitions 0:63 (ones-cols 0:64),
                    # so the custom-op recip runs aligned; the muls read the
                    # data rows 64:128 (standard-op partition shift)
                    rec = dv.tile([64, 2, 512], F32, tag="rec", name="rec")
                    nc.vector.reciprocal_approx_fast(rec[:, 0, :],
                                                     pv[0:64, 0, :])
                    nc.vector.tensor_mul(ot[0:64, p, :], pv[64:128, 0, :],
                                         rec[:, 0, :])
                    nc.vector.reciprocal_approx_fast(rec[:, 1, :],
                                                     pv[0:64, 1, :])
                    nc.vector.tensor_mul(ot[64:128, p, :], pv[64:128, 1, :],
                                         rec[:, 1, :])

                def emit_outproj(qb):
                    ot = ot_t.pop(qb)
                    for tcx in range(4):
                        pj = pvp.tile([128, 2, 512], F32, tag="pv",
                                      name="pj")
                        for n in range(2):
                            for p_ in range(NCOUT):
                                nc.tensor.matmul(
                                    pj[:, n, :],
                                    ot[:, p_, tcx * 128:(tcx + 1) * 128],
                                    wo_sb[:, p_, n * 512:(n + 1) * 512],
                                    start=(p_ == 0), stop=(p_ == NCOUT - 1))
                        oj = ojp.tile([128, 2, 512], F32, tag="oj", name="oj")
                        nc.scalar.copy(out=oj, in_=pj)
                        r0 = qb * 512 + tcx * 128
                        nc.default_dma_engine.dma_start(
                            out=out[r0:r0 + 128, :], in_=oj)

                emit_vproj(0)
                for i in range(LOOK):
                    emit_scores_exp(i)
                for i in range(NSTEP):
                    if i < NKC - 1:
                        emit_vproj(i + 1)
                    if i + LOOK < NSTEP:
                        emit_scores_exp(i + LOOK)
                    emit_pv(i)
                    qb, p, kc = sk(i)
                    if kc == NKC - 1:
                        emit_normalize(qb, p)
                        if p == NCOUT - 1:
                            emit_outproj(qb)
    nc.finalize()
    return nc


_CACHE = {}


def _get_runner():
    """Compile once per process; return f(in_maps) -> list of out dicts."""
    if "runner" in _CACHE:
        return _CACHE["runner"]
    import jax
    from jax.sharding import Mesh, PartitionSpec
    from jax.experimental.shard_map import shard_map
    from concourse import bass2jax

    nc = build_nc()
    bass2jax.install_neuronx_cc_hook()
    in_names, out_names, out_avals, zero_shapes = [], [], [], []
    for alloc in nc.m.functions[0].allocations:
        if not isinstance(alloc, mybir.MemoryLocationSet):
            continue
        name = alloc.memorylocations[0].name
        if alloc.kind == "ExternalInput":
            if name != "partition_id":
                in_names.append(name)
        elif alloc.kind == "ExternalOutput":
            out_names.append(name)
            shape = tuple(alloc.tensor_shape)
            dtype = mybir.dt.np(alloc.dtype)
            out_avals.append(jax.core.ShapedArray(shape, dtype))
            zero_shapes.append((shape, dtype))
    n_params = len(in_names)
    all_names = tuple(in_names + out_names)
    donate = tuple(range(n_params, n_params + len(out_names)))
    has_pid = nc.partition_id_tensor is not None

    def _body(*args):
        operands = list(args)
        names = all_names
        if has_pid:
            operands.append(bass2jax.partition_id_tensor())
            names = all_names + ("partition_id",)
        outs = bass2jax._bass_exec_p.bind(
            *operands, out_avals=tuple(out_avals), in_names=names,
            out_names=tuple(out_names), lowering_input_output_aliases=(),
            sim_require_finite=False, sim_require_nnan=False, nc=nc)
        return tuple(outs)

    devices = jax.devices()[:8]
    mesh = Mesh(np.asarray(devices), ("core",))
    specs = (PartitionSpec("core"),) * (n_params + len(out_names))
    f = jax.jit(shard_map(_body, mesh=mesh, in_specs=specs,
                          out_specs=(PartitionSpec("core"),) * len(out_names),
                          check_rep=False),
                donate_argnums=donate, keep_unused=True)

    def run(in_maps):
        concat_in = [np.concatenate([m[n] for m in in_maps], axis=0)
                     for n in in_names]
        concat_zeros = [np.zeros((8 * s[0], *s[1:]), d) for s, d in zero_shapes]
        outs = f(*concat_in, *concat_zeros)
        res = []
        for c in range(8):
            res.append({name: np.asarray(outs[i]).reshape(8, *out_avals[i].shape)[c]
                        for i, name in enumerate(out_names)})
        return res

    _CACHE["runner"] = run
    _CACHE["nc"] = nc
    return run


def make_in_maps(k, q, v, Wk, bk, Wq, bq, Wv, bv, Wo, bo):
    in_maps = []
    for c in range(8):
        b, g = divmod(c, 2)
        gs, ge = g * G, (g + 1) * G
        bqs = (bq[gs:ge] * SCALE).reshape(NCOUT, 128).T
        in_maps.append({
            "xq": np.ascontiguousarray(q[b].T).astype(ml_dtypes.bfloat16),
            "xk": np.ascontiguousarray(k[b].T).astype(ml_dtypes.bfloat16),
            "xv": np.ascontiguousarray(v[b].T).astype(ml_dtypes.bfloat16),
            "wq": np.ascontiguousarray(Wq[gs:ge, :].T * SCALE).astype(
                ml_dtypes.bfloat16),
            "wk": np.ascontiguousarray(Wk[gs:ge, :].T).astype(
                ml_dtypes.bfloat16),
            "wv": np.ascontiguousarray(Wv[gs:ge, :].T).astype(
                ml_dtypes.bfloat16),
            "wo": np.ascontiguousarray(Wo[:, gs:ge].T).astype(
                ml_dtypes.bfloat16),
            "bq": np.ascontiguousarray(bqs, dtype=np.float32),
        })
    return in_maps


def kernel(k, q, v, Wk, bk, Wq, bq, Wv, bv, Wo, bo):
    k = np.asarray(k, dtype=np.float32)
    q = np.asarray(q, dtype=np.float32)
    v = np.asarray(v, dtype=np.float32)
    Wk, bk = np.asarray(Wk, np.float32), np.asarray(bk, np.float32)
    Wq, bq = np.asarray(Wq, np.float32), np.asarray(bq, np.float32)
    Wv, bv = np.asarray(Wv, np.float32), np.asarray(bv, np.float32)
    Wo, bo = np.asarray(Wo, np.float32), np.asarray(bo, np.float32)

    in_maps = make_in_maps(k, q, v, Wk, bk, Wq, bq, Wv, bv, Wo, bo)
    run = _get_runner()
    res = run(in_maps)
    host_bias = (bo + Wo @ bv).astype(np.float32)
    out = np.empty((B, T, C), np.float32)
    for b in range(B):
        out[b] = res[2 * b]["out"] + res[2 * b + 1]["out"] + host_bias[None, :]
    return out


# revision 16
# speedup vs baseline: 1.1900x; 1.1900x over previous
"""Trainium2 Bass kernel for multi-head attention (B=4, T=2048, C=1024, H=16).

Sharding: 8 cores = (batch b in 0..3) x (head-group g in 0..1, 8 heads each).
Per core: QKV projections for its 512 dims, attention for 8 heads, partial
output projection. Host sums the two per-batch partials and adds the biases
that fold out of the device computation:
  - bk drops entirely (softmax is invariant to per-query additive constants)
  - bv folds to host:   out += Wo @ bv   (softmax rows sum to 1)
  - bo added on host
  - bq is applied on-device in the Q-projection drain (pre-scaled by host)

Numerics/engine plan (all projections bf16):
  - scores S^T = K^T.T Q^T per head pair, row-packed on the PE (the two
    64-contraction matmuls land in row groups 0/64 and run concurrently)
  - exp alternates whole key-chunks between ACT (exact Exp) and DVE
    (Schraudolph bit-trick: x -> int16(A*x+B) reinterpreted as bf16), so
    each psc PSUM buffer is freed by a single instruction
  - V is ones-augmented (cols 0:64 ones), so PV rows 0:63 hold the softmax
    denominator aligned for the custom-op reciprocal; muls read the data
    rows 64:128 with a standard-op partition shift
  - normalization on DVE, PSUM->SBUF drains split ACT/DVE, out-projection
    per query block, host adds partials
"""
import numpy as np
import ml_dtypes

import concourse.bass as bass
import concourse.mybir as mybir
import concourse.tile as tile
from concourse import bacc

F32 = mybir.dt.float32
BF16 = mybir.dt.bfloat16
I16 = mybir.dt.int16
AF = mybir.ActivationFunctionType
ALU = mybir.AluOpType

B, T, C = 4, 2048, 1024
H, CH = 16, 64
G = 512            # dims per head-group (8 heads)
NCIN = 8           # 128-chunks of C
NCOUT = 4          # 128-chunks of G
NTB = 4            # 512-wide t blocks
NKC = 16           # 128-wide key chunks
NQB = 4            # 512-wide query blocks
SCALE = 1.0 / np.sqrt(CH)

# Schraudolph exp: exp(x) ~= bitcast_bf16(int16(A_EXP*x + B_EXP)).
# Calibrated on the real logit distribution (std ~0.46); rel-err std ~1.8%,
# mean offset cancels in the softmax normalization.
A_EXP = float(2.0 ** 7 / np.log(2.0))
B_EXP = 16248.0


def build_nc(debug=False):
    nc = bacc.Bacc()
    xq = nc.declare_dram_parameter("xq", [C, T], BF16, isOutput=False)
    xk = nc.declare_dram_parameter("xk", [C, T], BF16, isOutput=False)
    xv = nc.declare_dram_parameter("xv", [C, T], BF16, isOutput=False)
    wq = nc.declare_dram_parameter("wq", [C, G], BF16, isOutput=False)
    wk = nc.declare_dram_parameter("wk", [C, G], BF16, isOutput=False)
    wv = nc.declare_dram_parameter("wv", [C, G], BF16, isOutput=False)
    wo = nc.declare_dram_parameter("wo", [G, C], BF16, isOutput=False)
    bq = nc.declare_dram_parameter("bq", [128, NCOUT], F32, isOutput=False)
    out = nc.declare_dram_parameter("out", [T, C], F32, isOutput=True)

    xq_r = xq.rearrange("(c p) t -> p c t", p=128)
    xk_r = xk.rearrange("(c p) t -> p c t", p=128)
    xv_r = xv.rearrange("(c p) t -> p c t", p=128)
    wk_r = wk.rearrange("(c p) g -> p c g", p=128)
    wq_r = wq.rearrange("(c p) g -> p c g", p=128)
    wv_r = wv.rearrange("(c p) g -> p c g", p=128)

    with tile.TileContext(nc) as tc:
        with tc.tile_pool(name="persist", bufs=1) as persist:
            qt = [persist.tile([128, T], BF16, tag=f"qt{i}", name=f"qt{i}")
                  for i in range(NCOUT)]
            kt = [persist.tile([128, T], BF16, tag=f"kt{i}", name=f"kt{i}")
                  for i in range(NCOUT)]
            # V augmented: per-head columns 0:64 are ones -> PV rows 0:63
            # all hold the softmax denominator, aligned for the custom-op
            # reciprocal (which reads inputs at the OUT base partition).
            # Whole-tile memset (contiguous); V-proj drains fill cols 64:128.
            v_aug = persist.tile([128, NKC, 8, 128], BF16, tag="vaug")
            nc.vector.memset(v_aug[:, :, :, :], 1.0)
            wo_sb = persist.tile([128, NCOUT, C], BF16, tag="wo")
            nc.scalar.dma_start(
                out=wo_sb, in_=wo.rearrange("(c p) g -> p c g", p=128))

            # ---------- scope 1: K and Q projections (bf16) ----------
            with tc.tile_pool(name="wkq", bufs=1) as wkq, \
                 tc.tile_pool(name="xsA", bufs=2) as xsA, \
                 tc.tile_pool(name="psA", bufs=4, space="PSUM") as psA:
                wk_sb = wkq.tile([128, NCIN, G], BF16, tag="wk")
                wq_sb = wkq.tile([128, NCIN, G], BF16, tag="wq")
                bq_sb = wkq.tile([128, NCOUT], F32, tag="bq")
                for ci in range(NCIN):
                    nc.scalar.dma_start(out=wk_sb[:, ci, :],
                                        in_=wk_r[:, ci, :])
                for tb in range(NTB):
                    xk_t = xsA.tile([128, NCIN, 512], BF16, tag="xstream")
                    for ci in range(NCIN):
                        nc.default_dma_engine.dma_start(
                            out=xk_t[:, ci, :],
                            in_=xk_r[:, ci, tb * 512:(tb + 1) * 512])
                    for co in range(NCOUT):
                        ps = psA.tile([128, 512], F32, tag="psA")
                        for ci in range(NCIN):
                            nc.tensor.matmul(
                                ps, wk_sb[:, ci, co * 128:(co + 1) * 128],
                                xk_t[:, ci, :],
                                start=(ci == 0), stop=(ci == NCIN - 1))
                        nc.vector.tensor_copy(
                            out=kt[co][:, tb * 512:(tb + 1) * 512], in_=ps)
                for ci in range(NCIN):
                    nc.scalar.dma_start(out=wq_sb[:, ci, :],
                                        in_=wq_r[:, ci, :])
                nc.scalar.dma_start(out=bq_sb, in_=bq[:, :])
                for tb in range(NTB):
                    xq_t = xsA.tile([128, NCIN, 512], BF16, tag="xstream")
                    for ci in range(NCIN):
                        nc.default_dma_engine.dma_start(
                            out=xq_t[:, ci, :],
                            in_=xq_r[:, ci, tb * 512:(tb + 1) * 512])
                    for co in range(NCOUT):
                        ps = psA.tile([128, 512], F32, tag="psA")
                        for ci in range(NCIN):
                            nc.tensor.matmul(
                                ps, wq_sb[:, ci, co * 128:(co + 1) * 128],
                                xq_t[:, ci, :],
                                start=(ci == 0), stop=(ci == NCIN - 1))
                        # wq/bq pre-scaled by 1/sqrt(dh) on the host
                        nc.scalar.activation(
                            qt[co][:, tb * 512:(tb + 1) * 512], ps,
                            AF.Identity, bias=bq_sb[:, co:co + 1])

            # ---------- scope 2: V projection overlapped with attention ----
            with tc.tile_pool(name="wv2", bufs=1) as wv2, \
                 tc.tile_pool(name="xsV", bufs=2) as xsV, \
                 tc.tile_pool(name="eb", bufs=6) as eb, \
                 tc.tile_pool(name="otp", bufs=2) as otp, \
                 tc.tile_pool(name="dv", bufs=2) as dv, \
                 tc.tile_pool(name="ojp", bufs=2) as ojp, \
                 tc.tile_pool(name="scp", bufs=2, space="PSUM") as scp, \
                 tc.tile_pool(name="pvp", bufs=2, space="PSUM") as pvp:
                wv_sb = wv2.tile([128, NCIN, G], BF16, tag="wv")
                for ci in range(NCIN):
                    nc.scalar.dma_start(out=wv_sb[:, ci, :],
                                        in_=wv_r[:, ci, :])

                # V projection, interleaved into the first attention
                # steps (one key-chunk ahead of its PV consumer)
                xv_tiles = {}

                def emit_vproj(tcix):
                    tb, sub = divmod(tcix, 4)
                    if sub == 0:
                        xv_t = xsV.tile([128, NCIN, 512], BF16,
                                        tag="xvstream", name="xv_t")
                        for ci in range(NCIN):
                            nc.default_dma_engine.dma_start(
                                out=xv_t[:, ci, :],
                                in_=xv_r[:, ci, tb * 512:(tb + 1) * 512])
                        xv_tiles[tb] = xv_t
                    xv_t = xv_tiles[tb]
                    ps2 = scp.tile([128, 2, 512], F32, tag="sc", name="psv")
                    ps = ps2[:, 0, :]
                    for ci in range(NCIN):
                        nc.tensor.matmul(
                            ps, xv_t[:, ci, sub * 128:(sub + 1) * 128],
                            wv_sb[:, ci, :],
                            start=(ci == 0), stop=(ci == NCIN - 1))
                    if tcix % 2 == 0:
                        nc.vector.tensor_copy(
                            out=v_aug[:, tcix, :, 64:128], in_=ps)
                    else:
                        nc.scalar.copy(
                            out=v_aug[:, tcix, :, 64:128], in_=ps)

                # attention, software-pipelined: scores/exp run LOOK steps
                # ahead of the PV accumulation over the flat (qb,p,kc) stream
                NSTEP = NQB * NCOUT * NKC
                LOOK = 2
                e_t, pv_t, ot_t = {}, {}, {}

                def sk(i):
                    return i >> 6, (i >> 4) & 3, i & 15

                def emit_scores_exp(i):
                    qb, p, kc = sk(i)
                    qsl = slice(qb * 512, (qb + 1) * 512)
                    ksl = slice(kc * 128, (kc + 1) * 128)
                    psc = scp.tile([128, 2, 512], F32, tag="sc", name="psc")
                    nc.tensor.matmul(psc[:, 0, :], kt[p][0:64, ksl],
                                     qt[p][0:64, qsl], start=True, stop=True)
                    nc.tensor.matmul(psc[:, 1, :], kt[p][64:128, ksl],
                                     qt[p][64:128, qsl], start=True, stop=True)
                    e = eb.tile([128, 2, 512], BF16, tag="e", name="e")
                    # whole-kc exp on a single engine so the psc buffer is
                    # freed by one instruction (decouples the ACT/DVE queues):
                    # odd kc -> DVE Schraudolph approx, else ACT exact
                    if kc % 2 == 1:
                        nc.vector.tensor_scalar(
                            e.bitcast(I16), psc, A_EXP, B_EXP,
                            ALU.mult, ALU.add)
                    else:
                        nc.scalar.activation(e, psc, AF.Exp)
                    e_t[i] = e

                def emit_pv(i):
                    qb, p, kc = sk(i)
                    if kc == 0:
                        pv_t[(qb, p)] = pvp.tile([128, 2, 512], F32,
                                                 tag="pv", name="pv")
                    pv = pv_t[(qb, p)]
                    e = e_t.pop(i)
                    nc.tensor.matmul(
                        pv[:, 0, :], v_aug[:, kc, 2 * p, :], e[:, 0, :],
                        start=(kc == 0), stop=(kc == NKC - 1))
                    nc.tensor.matmul(
                        pv[:, 1, :], v_aug[:, kc, 2 * p + 1, :], e[:, 1, :],
                        start=(kc == 0), stop=(kc == NKC - 1))

                def emit_normalize(qb, p):
                    if qb not in ot_t:
                        ot_t[qb] = otp.tile([128, NCOUT, 512], BF16,
                                            tag="ot", name="ot")
                    ot = ot_t[qb]
                    pv = pv_t.pop((qb, p))
                    # denominators sit on partitions 0:63 (ones-cols 0:64),
                    # so the custom-op recip runs aligned; the muls read the
                    # data rows 64:128 (standard-op partition shift)
                    rec = dv.tile([64, 2, 512], F32, tag="rec", name="rec")
                    nc.vector.reciprocal_approx_fast(rec[:, 0, :],
                                                     pv[0:64, 0, :])
                    nc.vector.tensor_mul(ot[0:64, p, :], pv[64:128, 0, :],
                                         rec[:, 0, :])
                    nc.vector.reciprocal_approx_fast(rec[:, 1, :],
                                                     pv[0:64, 1, :])
                    nc.vector.tensor_mul(ot[64:128, p, :], pv[64:128, 1, :],
                                         rec[:, 1, :])

                def emit_outproj(qb):
                    ot = ot_t.pop(qb)
                    for tcx in range(4):
                        pj = pvp.tile([128, 2, 512], F32, tag="pv",
                                      name="pj")
                        for n in range(2):
                            for p_ in range(NCOUT):
                                nc.tensor.matmul(
                                    pj[:, n, :],
                                    ot[:, p_, tcx * 128:(tcx + 1) * 128],
                                    wo_sb[:, p_, n * 512:(n + 1) * 512],
                                    start=(p_ == 0), stop=(p_ == NCOUT - 1))
                        oj = ojp.tile([128, 2, 512], F32, tag="oj", name="oj")
                        nc.scalar.copy(out=oj, in_=pj)
                        r0 = qb * 512 + tcx * 128
                        nc.default_dma_engine.dma_start(
                            out=out[r0:r0 + 128, :], in_=oj)

                emit_vproj(0)
                for i in range(LOOK):
                    emit_scores_exp(i)
                for i in range(NSTEP):
                    if i < NKC - 1:
                        emit_vproj(i + 1)
                    if i + LOOK < NSTEP:
                        emit_scores_exp(i + LOOK)
                    emit_pv(i)
                    qb, p, kc = sk(i)
                    if kc == NKC - 1:
                        emit_normalize(qb, p)
                        if p == NCOUT - 1:
                            emit_outproj(qb)
    nc.finalize()
    return nc


_CACHE = {}


def _get_runner():
    """Compile once per process; return f(in_maps) -> list of out dicts."""
    if "runner" in _CACHE:
        return _CACHE["runner"]
    import jax
    from jax.sharding import Mesh, PartitionSpec
    from jax.experimental.shard_map import shard_map
    from concourse import bass2jax

    nc = build_nc()
    bass2jax.install_neuronx_cc_hook()
    in_names, out_names, out_avals, zero_shapes = [], [], [], []
    for alloc in nc.m.functions[0].allocations:
        if not isinstance(alloc, mybir.MemoryLocationSet):
            continue
        name = alloc.memorylocations[0].name
        if alloc.kind == "ExternalInput":
            if name != "partition_id":
                in_names.append(name)
        elif alloc.kind == "ExternalOutput":
            out_names.append(name)
            shape = tuple(alloc.tensor_shape)
            dtype = mybir.dt.np(alloc.dtype)
            out_avals.append(jax.core.ShapedArray(shape, dtype))
            zero_shapes.append((shape, dtype))
    n_params = len(in_names)
    all_names = tuple(in_names + out_names)
    donate = tuple(range(n_params, n_params + len(out_names)))
    has_pid = nc.partition_id_tensor is not None

    def _body(*args):
        operands = list(args)
        names = all_names
        if has_pid:
            operands.append(bass2jax.partition_id_tensor())
            names = all_names + ("partition_id",)
        outs = bass2jax._bass_exec_p.bind(
            *operands, out_avals=tuple(out_avals), in_names=names,
            out_names=tuple(out_names), lowering_input_output_aliases=(),
            sim_require_finite=False, sim_require_nnan=False, nc=nc)
        return tuple(outs)

    devices = jax.devices()[:8]
    mesh = Mesh(np.asarray(devices), ("core",))
    specs = (PartitionSpec("core"),) * (n_params + len(out_names))
    f = jax.jit(shard_map(_body, mesh=mesh, in_specs=specs,
                          out_specs=(PartitionSpec("core"),) * len(out_names),
                          check_rep=False),
                donate_argnums=donate, keep_unused=True)

    def run(in_maps):
        concat_in = [np.concatenate([m[n] for m in in_maps], axis=0)
                     for n in in_names]
        concat_zeros = [np.zeros((8 * s[0], *s[1:]), d) for s, d in zero_shapes]
        outs = f(*concat_in, *concat_zeros)
        res = []
        for c in range(8):
            res.append({name: np.asarray(outs[i]).reshape(8, *out_avals[i].shape)[c]
                        for i, name in enumerate(out_names)})
        return res

    _CACHE["runner"] = run
    _CACHE["nc"] = nc
    return run


def make_in_maps(k, q, v, Wk, bk, Wq, bq, Wv, bv, Wo, bo):
    in_maps = []
    for c in range(8):
        b, g = divmod(c, 2)
        gs, ge = g * G, (g + 1) * G
        bqs = (bq[gs:ge] * SCALE).reshape(NCOUT, 128).T
        in_maps.append({
            "xq": np.ascontiguousarray(q[b].T).astype(ml_dtypes.bfloat16),
            "xk": np.ascontiguousarray(k[b].T).astype(ml_dtypes.bfloat16),
            "xv": np.ascontiguousarray(v[b].T).astype(ml_dtypes.bfloat16),
            "wq": np.ascontiguousarray(Wq[gs:ge, :].T * SCALE).astype(
                ml_dtypes.bfloat16),
            "wk": np.ascontiguousarray(Wk[gs:ge, :].T).astype(
                ml_dtypes.bfloat16),
            "wv": np.ascontiguousarray(Wv[gs:ge, :].T).astype(
                ml_dtypes.bfloat16),
            "wo": np.ascontiguousarray(Wo[:, gs:ge].T).astype(
                ml_dtypes.bfloat16),
            "bq": np.ascontiguousarray(bqs, dtype=np.float32),
        })
    return in_maps


def kernel(k, q, v, Wk, bk, Wq, bq, Wv, bv, Wo, bo):
    k = np.asarray(k, dtype=np.float32)
    q = np.asarray(q, dtype=np.float32)
    v = np.asarray(v, dtype=np.float32)
    Wk, bk = np.asarray(Wk, np.float32), np.asarray(bk, np.float32)
    Wq, bq = np.asarray(Wq, np.float32), np.asarray(bq, np.float32)
    Wv, bv = np.asarray(Wv, np.float32), np.asarray(bv, np.float32)
    Wo, bo = np.asarray(Wo, np.float32), np.asarray(bo, np.float32)

    in_maps = make_in_maps(k, q, v, Wk, bk, Wq, bq, Wv, bv, Wo, bo)
    run = _get_runner()
    res = run(in_maps)
    host_bias = (bo + Wo @ bv).astype(np.float32)
    out = np.empty((B, T, C), np.float32)
    for b in range(B):
        out[b] = res[2 * b]["out"] + res[2 * b + 1]["out"] + host_bias[None, :]
    return out
